# revision 56
# baseline (speedup 1.0000x reference)
# Trainium2 Bass kernel for nn_DinoDecoderBlock (B=8, NQ=NK=1024, C=768, H=12).
#
# Sharding: data-parallel over batch — batch element b runs on core b (8 cores,
# no collectives). Each core computes the full decoder block for its element.
#
# Device layout: feature-major activations [C, Ntok]; weights stationary.
# Speed tricks vs the bf16 baseline:
#  * Attention-side weight GEMMs + AV run in fp8e4 DoubleRow (2 contraction
#    chunks per matmul; weights host-scaled by WS=64, descale folded into
#    evictions). The MLP stays bf16 — it dominates the error budget.
#  * Softmax exp is a Schraudolph bit-trick: u8 = sat(EA*s + EB) bitcast as
#    fp8e4 IS exp(s*SCALE) to ~4%. One [128,1024] tensor_scalar/activation
#    per (head, key-chunk), alternating DVE/ACT — no table-exp anywhere, and
#    probs come out in fp8 ready for DoubleRow AV. Cross-attn mask folds into
#    the DVE intercept operand (fp16 mbias) or a tiny fp8 multiply.
#  * Softmax denominators come from a ones-column appended to V (row width
#    padded to VW=68 for DoubleRow stride alignment). Z is processed in
#    batches of 4 heads at partitions 0/32/64/96 of one tile: one batched
#    reciprocal + bf16 cast, PE broadcast, fused (oz*8)*zb eviction. The PE
#    part of each Z-batch is deferred into the next head's score window.
#  * K2/V2/Q2 GEMMs run as PE filler units inside attention windows / LN
#    boundaries; LN sums for x and y share one PSUM tile (rows 0 / 32) so
#    lny statistics overlap ln1's finish chain.
import numpy as np

B, NQ, NK, C, H = 8, 1024, 1024, 768, 12
HD = C // H          # 64
HID = 4 * C          # 3072
EPS = 1e-5
SCALE = HD ** -0.5
P = 128
FD = 512
KC = C // P          # 6 feature chunks
NKC = NK // P        # 8 key-token chunks
NQT = NQ // FD       # 2 query tiles
MQK = 2 * C // P     # 12 output chunks for fused Q,K
MH = HID // P        # 24
NPR = KC // 2        # 3 DoubleRow steps per C-contraction
VW = 68              # v65 row width: 64 vals + Z-ones col + 3 pad (H*VW%16==0)
WS = 64.0            # fp8 weight scale
OSC = 8.0            # o_fm extra scale (applied at epilogue)
EA = (8.0 / np.log(2.0)) * SCALE   # Schraudolph slope (folds SCALE)
EB = 55.63                         # Schraudolph intercept (tuned for e4m3)

_CACHE = {}


def _np_dt(dt):
    from concourse import mybir
    return np.dtype(mybir.dt.np(dt))


def _prep(inputs):
    """Host-side prep: fold LN gamma/beta into weights/biases, transpose to
    feature-major; attention weights to fp8e4 (x WS), MLP weights to bf16."""
    from concourse import mybir
    bf16 = _np_dt(mybir.dt.bfloat16)
    fp8 = _np_dt(mybir.dt.float8e4)
    f16 = np.float16
    f32 = np.float32

    g1 = np.asarray(inputs["ln1_g"], f32); b1 = np.asarray(inputs["ln1_b"], f32)
    g2 = np.asarray(inputs["ln2_g"], f32); b2 = np.asarray(inputs["ln2_b"], f32)
    g3 = np.asarray(inputs["ln3_g"], f32); b3 = np.asarray(inputs["ln3_b"], f32)
    gy = np.asarray(inputs["lny_g"], f32); by = np.asarray(inputs["lny_b"], f32)

    qkv_w = np.asarray(inputs["qkv_w"], f32)          # [2304, 768]
    wqk = qkv_w[: 2 * C] * g1[None, :]                # [1536, 768]
    wv = qkv_w[2 * C:] * g1[None, :]                  # [768, 768]
    bqk = qkv_w[: 2 * C] @ b1                         # [1536]
    bv = qkv_w[2 * C:] @ b1                           # [768]

    q_w = np.asarray(inputs["q_w"], f32)
    k_w = np.asarray(inputs["k_w"], f32)
    v_w = np.asarray(inputs["v_w"], f32)
    wq2 = q_w * g2[None, :]; bq2 = q_w @ b2
    wk2 = k_w * gy[None, :]; bk2 = k_w @ by
    wv2 = v_w * gy[None, :]; bv2 = v_w @ by

    fc1_w = np.asarray(inputs["fc1_w"], f32)
    wfc1 = fc1_w * g3[None, :]
    bfc1 = np.asarray(inputs["fc1_b"], f32) + fc1_w @ b3

    def as_bias_pm(vec):
        v = np.asarray(vec, f32)
        return np.ascontiguousarray(v.reshape(-1, P).T)

    def w8(mat):   # [in, out] fp8 with WS scale
        return np.ascontiguousarray(mat.T * WS).astype(fp8)

    def w16(mat):  # [in, out] bf16, unscaled
        return np.ascontiguousarray(mat.T).astype(bf16)

    mask = np.asarray(inputs["mask"]).astype(f32)
    maskT8 = np.ascontiguousarray(mask.T).astype(fp8)    # [NK, NQ] 0/1 fp8
    # Schraudolph intercept with mask folded (masked -> saturates u8 to 0)
    mbiasT = np.ascontiguousarray(
        (EB + (mask - 1.0) * 1000.0).T).astype(f16)      # [NK, NQ]

    bproj = np.asarray(inputs["attn_proj_b"], f32)
    bca = np.asarray(inputs["ca_proj_b"], f32)
    bfc2 = np.asarray(inputs["fc2_b"], f32)

    shared = {
        "wqkT": w8(wqk),                                            # [768,1536]
        "wvT": w8(wv),                                              # [768,768]
        "wprojT": w8(np.asarray(inputs["attn_proj_w"], f32)),
        "wq2T": w8(wq2),
        "wk2T": w8(wk2),
        "wv2T": w8(wv2),
        "wcaT": w8(np.asarray(inputs["ca_proj_w"], f32)),
        "wfc1T": w16(wfc1),                                         # [768,3072]
        "wfc2T": w16(np.asarray(inputs["fc2_w"], f32)),             # [3072,768]
        "bqk": as_bias_pm(bqk),
        "bq2": as_bias_pm(bq2),
        "bk2": as_bias_pm(bk2),
        "bproj": as_bias_pm(bproj),
        "bca": as_bias_pm(bca),
        "bfc1": as_bias_pm(bfc1),
        "bfc2": as_bias_pm(bfc2),
        "maskT8": maskT8,
        "mbiasT": mbiasT,
        "bv": np.ascontiguousarray(bv.reshape(1, C)),
        "bv2": np.ascontiguousarray(bv2.reshape(1, C)),
    }
    flags = {
        "use_bv": bool(np.any(bv != 0.0)),
        "use_bv2": bool(np.any(bv2 != 0.0)),
        "bz_proj": bool(np.all(bproj == 0.0)),
        "bz_ca": bool(np.all(bca == 0.0)),
        "bz_fc2": bool(np.all(bfc2 == 0.0)),
    }

    x = np.asarray(inputs["x"], f32)
    y = np.asarray(inputs["y"], f32)
    per_core = [{"xT": np.ascontiguousarray(x[b].T).astype(bf16),
                 "yT": np.ascontiguousarray(y[b].T).astype(bf16)}
                for b in range(B)]
    return shared, per_core, flags


def _emit(ctx, tc, nc, flags):
    from concourse import mybir

    f32 = mybir.dt.float32
    bf16 = mybir.dt.bfloat16
    f16 = mybir.dt.float16
    fp8 = mybir.dt.float8e4
    u8 = mybir.dt.uint8
    AF = mybir.ActivationFunctionType
    OP = mybir.AluOpType
    DR = mybir.MatmulPerfMode.DoubleRow

    # ---- DRAM I/O ----
    def din(name, shape, dt):
        return nc.dram_tensor(name, shape, dt, kind="ExternalInput").ap()

    xT = din("xT", [C, NQ], bf16)
    yT = din("yT", [C, NK], bf16)
    maskT8 = din("maskT8", [NK, NQ], fp8)
    mbiasT = din("mbiasT", [NK, NQ], f16)
    wqkT = din("wqkT", [C, 2 * C], fp8)
    wvT = din("wvT", [C, C], fp8)
    wprojT = din("wprojT", [C, C], fp8)
    wq2T = din("wq2T", [C, C], fp8)
    wk2T = din("wk2T", [C, C], fp8)
    wv2T = din("wv2T", [C, C], fp8)
    wcaT = din("wcaT", [C, C], fp8)
    wfc1T = din("wfc1T", [C, HID], bf16)
    wfc2T = din("wfc2T", [HID, C], bf16)
    bqk_d = din("bqk", [P, MQK], f32)
    bq2_d = din("bq2", [P, KC], f32)
    bk2_d = din("bk2", [P, KC], f32)
    bproj_d = din("bproj", [P, KC], f32)
    bca_d = din("bca", [P, KC], f32)
    bfc1_d = din("bfc1", [P, MH], f32)
    bfc2_d = din("bfc2", [P, KC], f32)
    bv_d = din("bv", [1, C], f32)
    bv2_d = din("bv2", [1, C], f32)
    xoutT = nc.dram_tensor("xoutT", [C, NQ], bf16,
                           kind="ExternalOutput").ap()

    def chunked(dram_ap, p=P):
        return dram_ap.rearrange("(kc p) m -> p kc m", p=p)

    # ---- long-lived pools ----
    const = ctx.enter_context(tc.tile_pool(name="const", bufs=1))
    masters = ctx.enter_context(tc.tile_pool(name="masters", bufs=2))
    stats = ctx.enter_context(tc.tile_pool(name="stats", bufs=2))
    wstream = ctx.enter_context(tc.tile_pool(name="wstream", bufs=2))
    psum = ctx.enter_context(tc.tile_pool(name="psum", bufs=2, space="PSUM"))
    # PSUM budget (8 banks): sc 2x[P,1024]f32 = 4, po 1x[P,1024]f32 = 2,
    # mm 2x[P,512]f32 = 2.

    # ---- constants ----
    ones_col = const.tile([P, 1], bf16)
    nc.vector.memset(ones_col, 1.0)
    ones_blk = const.tile([P, P], bf16)
    nc.vector.memset(ones_blk, 1.0)
    eps_t = const.tile([1, 1], f32)
    nc.vector.memset(eps_t, EPS)
    eb_t = const.tile([P, 1], f32)
    nc.vector.memset(eb_t, EB)

    def load_const(ap_dram, shape, cname):
        t = const.tile(shape, f32, name=cname, tag=cname)
        nc.sync.dma_start(out=t, in_=ap_dram)
        return t
    bqk_t = load_const(bqk_d, [P, MQK], "c_bqk")
    bq2_t = load_const(bq2_d, [P, KC], "c_bq2")
    bk2_t = load_const(bk2_d, [P, KC], "c_bk2")
    bproj_t = load_const(bproj_d, [P, KC], "c_bproj")
    bca_t = load_const(bca_d, [P, KC], "c_bca")
    bfc1_t = load_const(bfc1_d, [P, MH], "c_bfc1")
    bfc2_t = load_const(bfc2_d, [P, KC], "c_bfc2")
    bv_t = load_const(bv_d, [1, C], "c_bv") if flags["use_bv"] else None
    bv2_t = load_const(bv2_d, [1, C], "c_bv2") if flags["use_bv2"] else None

    def stream_w(dram_view):
        t = wstream.tile([P, KC, C], fp8, tag="wmov")
        kk, mm = dram_view.shape[1], dram_view.shape[2]
        nc.sync.dma_start(out=t[:, :kk, :mm], in_=dram_view)
        return t

    def stream_w16(dram_view):
        t = wstream.tile([P, KC, C], bf16, tag="wmov16")
        kk, mm = dram_view.shape[1], dram_view.shape[2]
        nc.sync.dma_start(out=t[:, :kk, :mm], in_=dram_view)
        return t

    # ---------------- LayerNorm (feature-major, interleavable) ----------
    class LN:
        """sum_x lands at PSUM row 0, sum_sq at row 32 of ONE tile, so two
        LNs' statistics can overlap through the 1-slot 'po' ring."""

        def __init__(self, get_chunk, ntok=NQ, tag="xn"):
            self.get_chunk = get_chunk
            self.ntok = ntok
            self.tag = tag
            self.nt = ntok // FD
            self.sums = psum.tile([P, ntok], f32, tag="po", bufs=1,
                                  name=f"sums_{tag}")
            self.done = 0
            self._st = None

        def chunk(self, tmp_pool):
            kc = self.done
            self.done += 1
            src = self.get_chunk(kc)
            sq = tmp_pool.tile([P, self.ntok], bf16, tag="sqb", bufs=2)
            nc.scalar.activation(sq, src, AF.Square)
            for t in range(self.nt):
                sl = slice(t * FD, (t + 1) * FD)
                nc.tensor.matmul(self.sums[0:1, sl], ones_col, src[:, sl],
                                 start=(kc == 0), stop=(kc == KC - 1))
                nc.tensor.matmul(self.sums[32:33, sl], ones_col, sq[:, sl],
                                 start=(kc == 0), stop=(kc == KC - 1))

        def finish_stats(self, tmp_pool):
            while self.done < KC:
                self.chunk(tmp_pool)
            ntok = self.ntok
            mean = stats.tile([1, ntok], f32, tag="st_mean", bufs=2)
            nc.vector.tensor_scalar_mul(mean, self.sums[0:1, :], 1.0 / C)
            m2 = stats.tile([1, ntok], f32, tag="st_scratch", bufs=2)
            nc.vector.tensor_tensor(m2, mean, mean, OP.mult)
            var = stats.tile([1, ntok], f32, tag="st_msq", bufs=2)
            nc.vector.scalar_tensor_tensor(var, self.sums[32:33, :], 1.0 / C,
                                           m2, OP.mult, OP.subtract)
            sd = stats.tile([1, ntok], f32, tag="st_sd", bufs=2)
            nc.scalar.activation(sd, var, AF.Sqrt, bias=eps_t)
            rstd = stats.tile([1, ntok], f32, tag="st_rstd", bufs=2)
            nc.vector.reciprocal_approx_fast(rstd, sd)
            meanb = stats.tile([1, ntok], bf16, tag="st_b16", bufs=4,
                               name="meanb")
            nc.vector.tensor_copy(meanb, mean)
            rstdb = stats.tile([1, ntok], bf16, tag="st_b16", bufs=4,
                               name="rstdb")
            nc.vector.tensor_copy(rstdb, rstd)
            self._st = (meanb, rstdb)

        def finish_apply(self, tmp_pool, out_pool, out_dt=None):
            from concourse import mybir as _mb
            out_dt = out_dt or _mb.dt.float8e4
            if self._st is None:
                self.finish_stats(tmp_pool)
            meanb, rstdb = self._st
            out = out_pool.tile([P, KC, self.ntok], out_dt, tag=self.tag,
                                bufs=1, name=f"ln_{self.tag}")
            # per-t broadcasts + evictions: consumers of the first half can
            # start while the second half is still evicting
            for t in range(self.nt):
                sl = slice(t * FD, (t + 1) * FD)
                mbt = psum.tile([P, FD], f32, tag="sc", bufs=4, name="mbt")
                nc.tensor.matmul(mbt, ones_blk[0:1, :], meanb[:, sl],
                                 start=True, stop=True)
                rbt = psum.tile([P, FD], f32, tag="sc", bufs=4, name="rbt")
                nc.tensor.matmul(rbt, ones_blk[0:1, :], rstdb[:, sl],
                                 start=True, stop=True)
                for kc in range(KC):
                    src = self.get_chunk(kc)
                    cen = tmp_pool.tile([P, FD], bf16, tag="cen", bufs=4)
                    nc.vector.tensor_tensor(cen, src[:, sl], mbt,
                                            OP.subtract)
                    nc.vector.tensor_tensor(out[:, kc, sl], cen, rbt,
                                            OP.mult)
            return out

        def finish(self, tmp_pool, out_pool, out_dt=None):
            self.finish_stats(tmp_pool)
            return self.finish_apply(tmp_pool, out_pool, out_dt)

    # ---------------- generic feature-major DoubleRow GEMM --------------
    def gemm_fm(w_of_m, xn, m_chunks, evict, per_m=None):
        for m in range(m_chunks):
            wt, co = w_of_m(m)
            for t in range(NQT):
                sl = slice(t * FD, (t + 1) * FD)
                pt = psum.tile([P, FD], f32, tag="mm", bufs=2)
                for j in range(NPR):
                    nc.tensor.matmul(pt, wt[:, 2 * j:2 * j + 2, co:co + P],
                                     xn[:, 2 * j:2 * j + 2, sl],
                                     start=(j == 0), stop=(j == NPR - 1),
                                     perf_mode=DR)
                evict(m, t, pt, sl)
            if per_m is not None:
                per_m(m)

    # ---------------- attention (single-head loop, N=1024 scores) -------
    def attention(qfm, kfm, v65, o_fm, mask_t, mbias_t, fillers=()):
        fillers = list(fillers)
        nfill = len(fillers)
        nslots = H * 4
        tick = [0]

        def fill_tick():
            tick[0] += 1
            while fillers and len(fillers) > (
                    nfill * (nslots - tick[0])) // nslots:
                fillers.pop(0)()

        zg = attn_pool.tile([97, NQ], f32, tag="zg", bufs=1, name="zg")
        nc.vector.memset(zg, 1.0)
        grp = []
        pending = []    # deferred PE part of Z batches

        def flush_group():
            if not grp:
                return
            zr = attn_pool.tile([97, NQ], f32, tag="zr97", bufs=1,
                                name="zr97")
            nc.vector.reciprocal_approx_fast(zr, zg)
            zrb = attn_pool.tile([97, NQ], bf16, tag="zrb97", bufs=1,
                                 name="zrb97")
            nc.vector.tensor_copy(zrb, zr)
            work = list(grp)
            grp.clear()

            def emit_pe():
                for gi, (h, oz) in enumerate(work):
                    kc_h, off = h // 2, (h % 2) * HD
                    gp = 32 * gi
                    for t in range(NQT):
                        sl = slice(t * FD, (t + 1) * FD)
                        zb = psum.tile([P, FD], f32, tag="mm", bufs=2,
                                       name="zb")
                        nc.tensor.matmul(zb[0:HD, :],
                                         ones_blk[gp:gp + 1, 0:HD],
                                         zrb[gp:gp + 1, sl],
                                         start=True, stop=True,
                                         tile_position=(gp, 0))
                        nc.vector.scalar_tensor_tensor(
                            o_fm[off:off + HD, kc_h, sl], oz[0:HD, sl],
                            OSC, zb[0:HD, :], OP.mult, OP.mult)
            pending.append(emit_pe)

        exp_k = [0]
        _dve_set = {0, 2, 5, 7, 9, 11, 13}       # 7/16 DVE-STT in cross
        _dve_mask = {14, 15}                     # 2 of ACT's masks on DVE

        def exp512(dst_u8, src_ps, nkc, sl):
            i = exp_k[0]
            exp_k[0] += 1
            if mbias_t is None:
                if i % 2 == 0:
                    nc.vector.tensor_scalar(dst_u8, src_ps, EA, EB,
                                            OP.mult, OP.add)
                else:
                    nc.scalar.activation(dst_u8, src_ps, AF.Identity,
                                         bias=eb_t[:, 0:1], scale=EA)
                return
            r = i % 16
            if r in _dve_set:
                nc.vector.scalar_tensor_tensor(dst_u8, src_ps, EA,
                                               mbias_t[:, nkc, sl],
                                               OP.mult, OP.add)
            else:
                nc.scalar.activation(dst_u8, src_ps, AF.Identity,
                                     bias=eb_t[:, 0:1], scale=EA)
                d8 = dst_u8.bitcast(fp8)
                eng = nc.vector if r in _dve_mask else nc.gpsimd
                eng.tensor_tensor(d8, d8, mask_t[:, nkc, sl], OP.mult)

        def emit_av(p, pr, po, h):
            pr8 = pr.bitcast(fp8)
            for t in range(NQT):
                sl = slice(t * FD, (t + 1) * FD)
                nc.tensor.matmul(po[0:VW, sl],
                                 v65[:, 2 * p:2 * p + 2, h, 0:VW],
                                 pr8[:, :, sl],
                                 start=(p == 0), stop=(p == NKC // 2 - 1),
                                 perf_mode=DR)

        for h in range(H):
            kc_h, off = h // 2, (h % 2) * HD
            po = psum.tile([P, NQ], f32, tag="po", bufs=1, name="po")
            pend = None
            for p in range(NKC // 2):
                pr = attn_pool.tile([P, 2, NQ], u8, tag="pr", bufs=3,
                                    name="pr")
                for jj in range(2):
                    nkc = 2 * p + jj
                    ksl = slice(nkc * P, (nkc + 1) * P)
                    for t in range(NQT):
                        sl = slice(t * FD, (t + 1) * FD)
                        ps = psum.tile([P, FD], f32, tag="sc", bufs=4,
                                       name="ps")
                        nc.tensor.matmul(ps,
                                         kfm[off:off + HD, kc_h, ksl],
                                         qfm[off:off + HD, kc_h, sl],
                                         start=True, stop=True)
                        exp512(pr[:, jj, sl], ps, nkc, sl)
                if pend is not None:
                    emit_av(*pend, po, h)
                if p == 1:
                    while pending:
                        pending.pop(0)()
                fill_tick()
                pend = (p, pr)
            emit_av(*pend, po, h)
            oz = attn_pool.tile([65, NQ], bf16, tag="oz", bufs=4, name="oz")
            nc.scalar.activation(oz, po[0:65, :], AF.Copy)
            gp = 32 * len(grp)
            nc.vector.tensor_copy(zg[gp:gp + 1, :], oz[64:65, :])
            grp.append((h, oz))
            if len(grp) == 4:
                flush_group()
        flush_group()
        while pending:
            pending.pop(0)()
        for f in fillers:
            f()

    # V GEMM as filler units: token-major V, ones col at HD, zero pad to VW
    def v_units(xn_st, wv_t, v65x):
        nc.vector.memset(v65x[:, :, :, HD:HD + 1], 1.0)
        nc.vector.memset(v65x[:, :, :, HD + 1:VW], 0.0)
        units = []
        for nkc in range(NKC):
            def ua(nkc=nkc):
                pva = psum.tile([P, FD], f32, tag="mm", bufs=2, name="vua")
                for j in range(NPR):
                    jsl = slice(2 * j, 2 * j + 2)
                    nc.tensor.matmul(pva,
                                     xn_st[:, jsl, nkc * P:(nkc + 1) * P],
                                     wv_t[:, jsl, 0:FD],
                                     start=(j == 0), stop=(j == NPR - 1),
                                     perf_mode=DR)
                nc.vector.tensor_scalar(
                    v65x[:, nkc, 0:8, 0:HD],
                    pva.rearrange("p (h d) -> p h d", d=HD),
                    1.0 / WS, None, OP.mult)

            def ub(nkc=nkc):
                pvb = psum.tile([P, FD], f32, tag="mm", bufs=2, name="vub")
                for j in range(NPR):
                    jsl = slice(2 * j, 2 * j + 2)
                    nc.tensor.matmul(pvb[:, 0:C - FD],
                                     xn_st[:, jsl, nkc * P:(nkc + 1) * P],
                                     wv_t[:, jsl, FD:C],
                                     start=(j == 0), stop=(j == NPR - 1),
                                     perf_mode=DR)
                nc.scalar.activation(
                    v65x[:, nkc, 8:12, 0:HD],
                    pvb[:, 0:C - FD].rearrange("p (h d) -> p h d", d=HD),
                    AF.Copy, scale=1.0 / WS)
            units += [ua, ub]
        return units

    def v_bias_add(v65x, bias_t):
        # runtime-unused fallback (biases are zero for this model config)
        bias_b = const.tile([1, C], bf16, name="bias_b", tag="c_biasb")
        nc.vector.tensor_copy(bias_b, bias_t)
        bbs = const.tile([P, C], f32, name="bbs", tag="c_bbs")
        for half, (a, b) in enumerate(((0, FD), (FD, C))):
            bbp = psum.tile([P, FD], f32, tag="mm", bufs=2, name="bbp")
            nc.tensor.matmul(bbp[:, 0:b - a], ones_blk[0:1, :],
                             bias_b[:, a:b], start=True, stop=True)
            nc.scalar.activation(bbs[:, a:b], bbp[:, 0:b - a], AF.Copy)
        bbs_h = bbs.rearrange("p (h d) -> p h d", d=HD)
        for nkc in range(NKC):
            nc.vector.tensor_tensor(v65x[:, nkc, :, 0:HD],
                                    v65x[:, nkc, :, 0:HD], bbs_h, OP.add)

    # ================= program =================
    x0 = masters.tile([P, KC, NQ], bf16, tag="xmaster")
    xT_ch = chunked(xT)
    _qs = [nc.sync, nc.gpsimd, nc.scalar]
    for _kc in range(KC):
        _qs[_kc % 3].dma_start(out=x0[:, _kc, 0:FD], in_=xT_ch[:, _kc, 0:FD])
        _qs[(_kc + 1) % 3].dma_start(out=x0[:, _kc, FD:NQ],
                                     in_=xT_ch[:, _kc, FD:NQ])
    xT_y = chunked(yT)

    with tc.tile_pool(name="attn", bufs=1) as attn_pool:
        v65_s = attn_pool.tile([P, NKC, H, VW], fp8, tag="v65s", bufs=1)
        v65_c = attn_pool.tile([P, NKC, H, VW], fp8, tag="v65c", bufs=1)
        # ---- phase A ----
        with tc.tile_pool(name="phA", bufs=2) as pa:
            # y chunks DMA'd up-front (overlaps x/weight DMA and LN1)
            ystr = []
            for kc in range(KC):
                t = pa.tile([P, NK], bf16, tag="ystr", bufs=KC,
                            name=f"ystr{kc}")
                (nc.gpsimd if kc % 2 == 0 else nc.scalar).dma_start(
                    out=t, in_=xT_y[:, kc, :])
                ystr.append(t)

            ln1 = LN(lambda kc: x0[:, kc, :], tag="xn")
            for kc in range(KC):
                ln1.chunk(pa)
            ln1.finish_stats(pa)
            lny = LN(lambda kc: ystr[kc], ntok=NK, tag="yn")
            for kc in range(KC):
                lny.chunk(pa)          # overlaps ln1 finish chain
            xn1 = ln1.finish_apply(pa, pa)
            lny.finish_stats(pa)

            qfm = attn_pool.tile([P, KC, NQ], fp8, tag="qfm", bufs=1)
            kfm = attn_pool.tile([P, KC, NQ], fp8, tag="kfm", bufs=1)
            wq_half = stream_w(chunked(wqkT)[:, :, 0:C])
            wk_half = stream_w(chunked(wqkT)[:, :, C:2 * C])

            def qk_evict(m, t, pt, sl):
                dst = qfm if m < KC else kfm
                nc.scalar.activation(dst[:, m % KC, sl], pt, AF.Identity,
                                     bias=bqk_t[:, m:m + 1], scale=1.0 / WS)
            gemm_fm(lambda m: (wq_half, m * P) if m < KC
                    else (wk_half, (m - KC) * P), xn1, MQK, qk_evict)
            yn = lny.finish_apply(pa, attn_pool)

            wv_t = stream_w(chunked(wvT))
            for u in v_units(xn1, wv_t, v65_s):
                u()
            if flags["use_bv"]:
                v_bias_add(v65_s, bv_t)

        # ---- phase B: self-attention (+ V2 fillers) + proj ----
        with tc.tile_pool(name="phB", bufs=2) as pb:
            o_fm = attn_pool.tile([P, KC, NQ], fp8, tag="ofm", bufs=1)
            units = []
            if not flags["use_bv2"]:
                wv2_t = stream_w(chunked(wv2T))
                units += v_units(yn, wv2_t, v65_c)

            x1 = masters.tile([P, KC, NQ], bf16, tag="xmaster")
            wproj_t = stream_w(chunked(wprojT))

            attention(qfm, kfm, v65_s, o_fm, None, None, fillers=units)

            ln2 = LN(lambda kc: x1[:, kc, :], tag="ln2")

            def proj_evict(m, t, pt, sl):
                nc.vector.scalar_tensor_tensor(
                    x1[:, m, sl], pt, 1.0 / (WS * OSC),
                    x0[:, m, sl], OP.mult, OP.add)
                if not flags["bz_proj"]:
                    nc.vector.tensor_scalar_add(x1[:, m, sl], x1[:, m, sl],
                                                bproj_t[:, m:m + 1])
            gemm_fm(lambda m: (wproj_t, m * P), o_fm, KC, proj_evict,
                    per_m=lambda m: ln2.chunk(pb))

        # ---- phase C: cross-attention + proj ----
        with tc.tile_pool(name="phC", bufs=2) as pc:
            if flags["use_bv2"]:
                wv2_t = stream_w(chunked(wv2T))
                for u in v_units(yn, wv2_t, v65_c):
                    u()
                v_bias_add(v65_c, bv2_t)

            mask_t = pc.tile([P, NKC, NQ], fp8, tag="mask8", bufs=1)
            nc.sync.dma_start(out=mask_t,
                              in_=maskT8.rearrange("(kc p) n -> p kc n", p=P))
            mbias_t = pc.tile([P, NKC, NQ], f16, tag="mbias", bufs=1)
            nc.gpsimd.dma_start(out=mbias_t,
                                in_=mbiasT.rearrange("(kc p) n -> p kc n",
                                                     p=P))

            ln2.finish_stats(pc)
            # K2 GEMM fills the PE while ln2's broadcast/eviction chain runs
            k2 = attn_pool.tile([P, KC, NK], fp8, tag="k2", bufs=1)
            wk2_t = stream_w(chunked(wk2T))

            def k2_evict(m, t, pt, sl):
                nc.scalar.activation(k2[:, m, sl], pt, AF.Identity,
                                     bias=bk2_t[:, m:m + 1], scale=1.0 / WS)
            gemm_fm(lambda m: (wk2_t, m * P), yn, KC, k2_evict)

            xn2 = ln2.finish_apply(pc, attn_pool)

            q2 = attn_pool.tile([P, KC, NQ], fp8, tag="qfm", bufs=1)
            wq2_t = stream_w(chunked(wq2T))

            def q2_unit(m):
                def run():
                    for t in range(NQT):
                        sl = slice(t * FD, (t + 1) * FD)
                        pt = psum.tile([P, FD], f32, tag="mm", bufs=2,
                                       name="q2pt")
                        for j in range(NPR):
                            nc.tensor.matmul(
                                pt, wq2_t[:, 2 * j:2 * j + 2,
                                          m * P:(m + 1) * P],
                                xn2[:, 2 * j:2 * j + 2, sl],
                                start=(j == 0), stop=(j == NPR - 1),
                                perf_mode=DR)
                        nc.scalar.activation(q2[:, m, sl], pt, AF.Identity,
                                             bias=bq2_t[:, m:m + 1],
                                             scale=1.0 / WS)
                return run
            q2_unit(0)()
            q2_unit(1)()
            q2_fillers = [q2_unit(m) for m in range(2, KC)]

            o2_fm = attn_pool.tile([P, KC, NQ], fp8, tag="ofm", bufs=1)
            x2 = masters.tile([P, KC, NQ], bf16, tag="xmaster")
            wca_t = stream_w(chunked(wcaT))

            attention(q2, k2, v65_c, o2_fm, mask_t, mbias_t,
                      fillers=q2_fillers)

            ln3 = LN(lambda kc: x2[:, kc, :], tag="ln3")

            def ca_evict(m, t, pt, sl):
                nc.vector.scalar_tensor_tensor(
                    x2[:, m, sl], pt, 1.0 / (WS * OSC),
                    x1[:, m, sl], OP.mult, OP.add)
                if not flags["bz_ca"]:
                    nc.vector.tensor_scalar_add(x2[:, m, sl], x2[:, m, sl],
                                                bca_t[:, m:m + 1])
            gemm_fm(lambda m: (wca_t, m * P), o2_fm, KC, ca_evict,
                    per_m=lambda m: ln3.chunk(pc))
            # prefetch fc1 quarter-0 weights: overlaps ln3 finish
            w1q0 = stream_w16(chunked(wfc1T)[:, :, 0:KC * P])

    # ---- phase D: MLP (bf16 — dominates the fp8 error budget) ----
    with tc.tile_pool(name="phD", bufs=2) as pd:
        xn3 = ln3.finish(pd, pd, out_dt=bf16)

        h1 = pd.tile([P, MH, NQ], bf16, tag="h1", bufs=1)
        w1view = chunked(wfc1T)
        for quarter in range(4):
            w1q = w1q0 if quarter == 0 else stream_w16(
                w1view[:, :, quarter * KC * P:(quarter + 1) * KC * P])
            for mi in range(KC):
                m = quarter * KC + mi
                for t in range(NQT):
                    sl = slice(t * FD, (t + 1) * FD)
                    pt = psum.tile([P, FD], f32, tag="mm", bufs=2)
                    for kc in range(KC):
                        nc.tensor.matmul(
                            pt, w1q[:, kc, mi * P:(mi + 1) * P],
                            xn3[:, kc, sl],
                            start=(kc == 0), stop=(kc == KC - 1))
                    nc.scalar.activation(h1[:, m, sl], pt, AF.Gelu,
                                         bias=bfc1_t[:, m:m + 1])

        xout = masters.tile([P, KC, NQ], bf16, tag="xmaster")
        xout_dram = xoutT.rearrange("(kc p) m -> p kc m", p=P)
        w2view = chunked(wfc2T)  # [128, 24, 768]

        def stream_w_pd(view):
            t = pd.tile([P, KC, C], bf16, tag="w2", bufs=4, name="w2t")
            nc.sync.dma_start(out=t, in_=view)
            return t
        w2tiles = [(stream_w_pd(w2view[:, h * 12:h * 12 + KC, :]),
                    stream_w_pd(w2view[:, h * 12 + KC:h * 12 + 12, :]))
                   for h in range(2)]
        for half in range(2):
            w2a, w2b = w2tiles[half]
            for m in range(KC):
                for t in range(NQT):
                    sl = slice(t * FD, (t + 1) * FD)
                    pt = psum.tile([P, FD], f32, tag="mm", bufs=2)
                    for kq in range(12):
                        w2t = w2a if kq < KC else w2b
                        nc.tensor.matmul(
                            pt, w2t[:, kq % KC, m * P:(m + 1) * P],
                            h1[:, half * 12 + kq, sl],
                            start=(kq == 0), stop=(kq == 11))
                    if half == 0:
                        nc.vector.tensor_tensor(
                            xout[:, m, sl], pt, x2[:, m, sl], OP.add)
                        if not flags["bz_fc2"]:
                            nc.vector.tensor_scalar_add(
                                xout[:, m, sl], xout[:, m, sl],
                                bfc2_t[:, m:m + 1])
                    else:
                        nc.vector.tensor_tensor(
                            xout[:, m, sl], pt, xout[:, m, sl], OP.add)
                if half == 1:
                    nc.sync.dma_start(out=xout_dram[:, m, :],
                                      in_=xout[:, m, :])


def _build(flags):
    import concourse.bacc as bacc
    import concourse.tile as tile
    from contextlib import ExitStack

    nc = bacc.Bacc("TRN2", target_bir_lowering=False, debug=False)
    with tile.TileContext(nc) as tc, ExitStack() as ctx:
        _emit(ctx, tc, nc, flags)
    nc.compile()
    return nc


def kernel(**inputs):
    from concourse.bass_utils import run_bass_kernel_spmd

    shared, per_core, flags = _prep(inputs)
    key = tuple(sorted(flags.items()))
    if key not in _CACHE:
        _CACHE[key] = _build(flags)
    nc = _CACHE[key]

    in_maps = []
    for b in range(B):
        m = dict(shared)
        m.update(per_core[b])
        in_maps.append(m)
    res = run_bass_kernel_spmd(nc, in_maps, core_ids=list(range(B)))
    x_out = np.stack([np.ascontiguousarray(np.asarray(r["xoutT"]).T)
                      for r in res.results]).astype(np.float32)
    y_out = np.asarray(inputs["y"], np.float32)
    return (x_out, y_out)
